# revision 43
# baseline (speedup 1.0000x reference)
"""Trainium2 Bass kernel for paged causal self-attention (GQA + YaRN rope).

Sharding: tensor-parallel over heads. Core c (of 8) owns kv-head c and
q-heads 2c, 2c+1 for both batches. Each core computes a partial output
y_c = attn_c @ Wo_c.T over its 256 channels; the host sums the 8 partials.

The reference's scatter of new K/V into the pools is dead code w.r.t. the
returned output; new K/V are consumed directly from SBUF. The past-KV
gather (slot_map indexed) and the [s,d]/[d,s] layout transposes are done
on the host, so the device sees two contiguous fp16 layouts.

All matmul operands are fp16 (host-precast); accumulation stays fp32 in
PSUM. exp uses a constant -4 bias (cancels in softmax) for fp16 range
margin. Causal diagonal chunks are column-sliced and share one [128,128]
triangular mask tile.
"""

import sys

sys.path.insert(0, "/opt/trn_rl_repo")

import ml_dtypes
import numpy as np

NP_BF16 = np.dtype(ml_dtypes.bfloat16)

import concourse.bacc as bacc
import concourse.bass as bass
import concourse.tile as tile
from concourse import mybir
from concourse.bass_utils import run_bass_kernel_spmd

F32 = mybir.dt.float32
F16 = mybir.dt.float16
BF16 = mybir.dt.bfloat16
# stationary matmul operands use bf16 (enables fast weight load);
# moving operands stay fp16 for precision
STAT = BF16
EXP = mybir.ActivationFunctionType.Exp

B, T, PAST = 2, 1024, 1024
H, HKV, D = 16, 8, 128
G = H // HKV            # q heads per kv head
C = H * D               # 2048
TOTAL = PAST + T        # 2048
NB = B * T              # 2048 flattened tokens
NCORES = 8
P = 128
TB = 512                # token block
NEG = -60000.0          # mask value (fp16-representable; exp underflows to 0)
EBIAS = -4.0            # constant exp bias; cancels in softmax


def _emit(tc, io):
    nc = tc.nc
    (xT, wq, wk, wv, wo, kpT, vpn, cosq, sinq, cosk, sink, tri, rperm,
     ones, ident, y) = io

    with (
        tc.tile_pool(name="const", bufs=1) as cp,
        tc.tile_pool(name="persist", bufs=1) as pp,
        tc.tile_pool(name="ysb", bufs=4) as yp,
        tc.tile_pool(name="xt", bufs=8) as xp,
        tc.tile_pool(name="rope", bufs=3) as rp,
        tc.tile_pool(name="exps", bufs=2) as ep,
        tc.tile_pool(name="sums", bufs=2) as sp_,
        tc.tile_pool(name="attw", bufs=2) as aw,
    ):
        # ---- weight tiles (loaded chunked, interleaved with the first x
        # tiles, inside the phase-1 loop) ----
        wq_t = pp.tile([P, 16, G * P], STAT)
        wk_t = pp.tile([P, 16, P], STAT)
        wv_t = pp.tile([P, 16, P], STAT)

        ebias = cp.tile([P, 1], F32)
        nc.vector.memset(ebias[:], EBIAS)

        # ---- remaining constant tiles; their loads are emitted inside the
        # first phase-1 block (SWDGE on the idle GpSimd queue, dep-delayed
        # behind the last weight chunk so they don't steal DMA bandwidth
        # from the startup-critical loads) ----
        gdep = cp.tile([P, 8], STAT)
        cosq_t = pp.tile([P, T], F16)
        sinq_t = pp.tile([P, T], F16)
        cosk_t = pp.tile([P, T], F16)
        sink_t = pp.tile([P, T], F16)
        tri_t = cp.tile([P, P], F16)
        rperm_t = cp.tile([P, P], STAT)
        ones_t = cp.tile([P, P], STAT)
        ident_t = cp.tile([P, P], STAT)
        kT_past = pp.tile([P, B, 8, P], STAT)  # [d, b, chunk, s%128]
        vg = pp.tile([P, B, 8, P], STAT)       # [s%128, b, chunk, d]
        wo_t = pp.tile([P, G, C], F16)

        gdep2 = cp.tile([P, 8], STAT)

        def emit_const_loads_a():
            # phase-1 constants (rope tables etc.) — gated behind the last
            # weight chunk so they don't contend with the startup loads
            nc.gpsimd.tensor_copy(gdep[:], wq_t[:, 15, 0:8])
            nc.gpsimd.dma_start(cosq_t[:], cosq[:])
            nc.gpsimd.dma_start(sinq_t[:], sinq[:])
            nc.gpsimd.dma_start(cosk_t[:], cosk[:])
            nc.gpsimd.dma_start(sink_t[:], sink[:])
            nc.gpsimd.dma_start(tri_t[:], tri[:])
            nc.gpsimd.dma_start(rperm_t[:], rperm[:])
            nc.gpsimd.dma_start(ones_t[:], ones[:])
            nc.gpsimd.dma_start(ident_t[:], ident[:])

        def emit_const_loads_b(gate):
            # phase-2 constants — gated to the third projection block so
            # their 2MB doesn't clog the DMA rings during phase-1 streaming
            nc.gpsimd.tensor_copy(gdep2[:], gate[:, 0:8])
            nc.gpsimd.dma_start(kT_past[:],
                                kpT.rearrange("p (b j m) -> p b j m", b=B, j=8))
            nc.gpsimd.dma_start(vg[:],
                                vpn.rearrange("p (b j m) -> p b j m", b=B, j=8))
            nc.gpsimd.dma_start(wo_t[:], wo.rearrange("p (g m) -> p g m", g=G))

        # ---- persistent activations ----
        qT0 = pp.tile([P, NB], F16)       # q head 2c,   [d, token]  (moving)
        qT1 = pp.tile([P, NB], F16)       # q head 2c+1
        kT_new = pp.tile([P, NB], STAT)   # new keys,    [d, token]  (stationary)
        v_nat = pp.tile([P, B, 8, P], STAT)    # new values, [t%128, b, chunk, d]
        att0 = pp.tile([P, NB], STAT)     # attention out head 2c, [d, token]
        att1 = pp.tile([P, NB], STAT)

        # ================= phase 1: projections + rope =================
        with (
            tc.tile_pool(name="pproj", bufs=1, space="PSUM") as pjp,
            tc.tile_pool(name="prope", bufs=2, space="PSUM") as rpp,
            tc.tile_pool(name="ptr", bufs=1, space="PSUM") as trp,
        ):
            # Rope/v-transpose emission is deferred one block: the PE's
            # in-order stream gets block N+1's projection matmuls BEFORE
            # block N's rope matmuls, so PSUM-evacuation copies (ACT/DVE)
            # have a full block of slack to land.
            pending = None

            def emit_rope(state):
                tbp, raw_k_, raw_q0_, raw_q1_, vsb_ = state
                n0p = tbp * TB
                bp = tbp // 2
                tposp = (tbp % 2) * TB
                # v transposes first (their input was evacuated earliest);
                # all four land in one PSUM tile, one batched evacuation
                vtp = trp.tile([P, 4, P], STAT, name="vtp", tag="vtp")
                for j4 in range(TB // P):
                    nc.tensor.transpose(vtp[:, j4, :],
                                        vsb_[:, j4 * P:(j4 + 1) * P], ident_t[:])
                # evacuate the transposes first so their PSUM bank (reused by
                # phase-2 pools) is released before the mul chain drains
                nc.vector.tensor_copy(
                    v_nat[:, bp, (tbp % 2) * 4:(tbp % 2) * 4 + 4, :], vtp[:])
                for raw, dst, ct, stt in (
                    (raw_k_, kT_new, cosk_t, sink_t),
                    (raw_q0_, qT0, cosq_t, sinq_t),
                    (raw_q1_, qT1, cosq_t, sinq_t),
                ):
                    rot = rpp.tile([P, TB], F32, name="rot", tag="rot")
                    nc.tensor.matmul(rot[:], rperm_t[:], raw[:], start=True,
                                     stop=True)
                    t1 = rp.tile([P, TB], F16, name="t1", tag="t1")
                    nc.vector.tensor_mul(t1[:], raw[:], ct[:, tposp:tposp + TB])
                    t2 = rp.tile([P, TB], F16, name="t2", tag="t2")
                    nc.vector.tensor_mul(t2[:], rot[:], stt[:, tposp:tposp + TB])
                    nc.vector.tensor_add(dst[:, n0p:n0p + TB], t1[:], t2[:])

            # block order (2,3,0,1): the last-roped block (1) is only needed
            # by the final attention groups, so phase 2 starts without
            # waiting for the phase-1 tail.
            for ti, tb in enumerate((2, 3, 0, 1)):
                n0 = tb * TB

                # previous block's q psum evacuations (ACT queue, ready now)
                if pending is not None:
                    tbp, q0p_, q1p_, raw_k_, vsb_ = pending
                    raw_q0 = rp.tile([P, TB], F16, name="raw_q0", tag="rawq0")
                    nc.scalar.copy(raw_q0[:], q0p_[:])
                    raw_q1 = rp.tile([P, TB], F16, name="raw_q1", tag="rawq1")
                    nc.scalar.copy(raw_q1[:], q1p_[:])
                    pending = (tbp, raw_k_, raw_q0, raw_q1, vsb_)

                q0p = pjp.tile([P, TB], F32, name="q0p", tag="q0")
                q1p = pjp.tile([P, TB], F32, name="q1p", tag="q1")
                kkp = pjp.tile([P, TB], F32, name="kkp", tag="kk")
                vvp = pjp.tile([P, TB], F32, name="vvp", tag="vv")
                for kcg in range(4):
                    if ti == 0:
                        # first block: interleave the weight-chunk loads with
                        # the x stream so the first matmul starts early
                        nc.sync.dma_start(
                            wq_t[:, 4 * kcg:4 * kcg + 4, :],
                            wq[:, kcg * 4 * G * P:(kcg + 1) * 4 * G * P]
                            .rearrange("p (kc m) -> p kc m", kc=4))
                        nc.sync.dma_start(
                            wk_t[:, 4 * kcg:4 * kcg + 4, :],
                            wk[:, kcg * 4 * P:(kcg + 1) * 4 * P]
                            .rearrange("p (kc m) -> p kc m", kc=4))
                        nc.sync.dma_start(
                            wv_t[:, 4 * kcg:4 * kcg + 4, :],
                            wv[:, kcg * 4 * P:(kcg + 1) * 4 * P]
                            .rearrange("p (kc m) -> p kc m", kc=4))
                    xt4 = xp.tile([P, 4, TB], F16, name="xt4", tag="xt")
                    nc.sync.dma_start(xt4[:], xT[:, tb, 4 * kcg:4 * kcg + 4, :])
                    for kc4 in range(4):
                        kc = 4 * kcg + kc4
                        xt = xt4[:, kc4, :]
                        st = (kc == 0)
                        sp = (kc == 15)
                        nc.tensor.matmul(q0p[:], wq_t[:, kc, 0:P], xt, start=st, stop=sp)
                        nc.tensor.matmul(q1p[:], wq_t[:, kc, P:2 * P], xt, start=st, stop=sp)
                        nc.tensor.matmul(kkp[:], wk_t[:, kc, :], xt, start=st, stop=sp)
                        nc.tensor.matmul(vvp[:], wv_t[:, kc, :], xt, start=st, stop=sp)

                # evacuate kk/vv immediately on separate engines
                raw_k = rp.tile([P, TB], F16, name="raw_k", tag="rawk")
                nc.scalar.copy(raw_k[:], kkp[:])
                vsb = rp.tile([P, TB], STAT, name="vsb", tag="vsb")
                nc.vector.tensor_copy(vsb[:], vvp[:])

                if ti == 0:
                    emit_const_loads_a()
                elif ti == 2:
                    emit_const_loads_b(raw_k)
                if pending is not None:
                    emit_rope(pending)
                pending = (tb, q0p, q1p, raw_k, vsb)

            # drain the final block (q0/q1 evacuations on separate engines —
            # no later projection matmuls will hide their latency)
            tbp, q0p_, q1p_, raw_k_, vsb_ = pending
            raw_q0 = rp.tile([P, TB], F16, name="raw_q0", tag="rawq0")
            nc.scalar.copy(raw_q0[:], q0p_[:])
            raw_q1 = rp.tile([P, TB], F16, name="raw_q1", tag="rawq1")
            nc.vector.tensor_copy(raw_q1[:], q1p_[:])
            emit_rope((tbp, raw_k_, raw_q0, raw_q1, vsb_))

        # ================= phase 2+3: attention + output proj =================
        with (
            tc.tile_pool(name="pscore", bufs=3, space="PSUM") as scp,
            tc.tile_pool(name="pav", bufs=2, space="PSUM") as avp,
            tc.tile_pool(name="pbc", bufs=1, space="PSUM") as bcp,
            tc.tile_pool(name="py", bufs=2, space="PSUM") as pyp,
        ):
            def emit_outproj(t0_):
                # output projection for 512 tokens; evacuate the four 512-ch
                # chunks into one wide tile, one contiguous DMA per 128 toks
                for tc4 in range(4):
                    tt0 = t0_ + tc4 * P
                    ysb = yp.tile([P, 4, TB], F16, name="ysbt", tag="ysbt")
                    for cb in range(4):
                        yps = pyp.tile([P, TB], F32, name="yps", tag="y")
                        nc.tensor.matmul(yps[:], att0[:, tt0:tt0 + P],
                                         wo_t[:, 0, cb * TB:(cb + 1) * TB],
                                         start=True, stop=False)
                        nc.tensor.matmul(yps[:], att1[:, tt0:tt0 + P],
                                         wo_t[:, 1, cb * TB:(cb + 1) * TB],
                                         start=False, stop=True)
                        if (tc4 + cb) % 2 == 0:
                            nc.scalar.copy(ysb[:, cb, :], yps[:])
                        else:
                            nc.vector.tensor_copy(ysb[:, cb, :], yps[:])
                    nc.sync.dma_start(y[tt0:tt0 + P, :], ysb[:])

            pend_out = None
            for b in range(B):
                for tbq in range(2):             # query block of 512 in batch
                    t0 = b * T + tbq * TB
                    for g, (qT, att) in enumerate(((qT0, att0), (qT1, att1))):
                        if g == 1 and pend_out is not None:
                            # previous block's output projection, emitted
                            # here so its att inputs have a chunk-loop of
                            # slack and its matmuls fill exp-latency bubbles
                            emit_outproj(pend_out)
                            pend_out = None
                        q_ap = qT[:, t0:t0 + TB]
                        njnew = 4 * tbq + 4
                        nch = 8 + njnew

                        # chunk list: past (full), sliced-diag new (desc ri),
                        # then full new; last chunk is full-width.
                        chunks = [(kT_past[:, b, j, :], vg[:, b, j, :], None)
                                  for j in range(8)]
                        sliced = []
                        full_new = []
                        for j in range(njnew):
                            koff = b * T + j * P
                            ri = j - 4 * tbq
                            ent = (kT_new[:, koff:koff + P],
                                   v_nat[:, b, j, :], ri if ri > 0 else None,
                                   ri == 0)
                            if ri > 0:
                                sliced.append(ent)
                            else:
                                full_new.append(ent)
                        sliced.reverse()   # descending ri
                        # order: past, sliced (ri 3..1), full new (ri<0), ri==0 last
                        full_new.sort(key=lambda e: e[3])
                        chunks = ([(k_, v_, None, False) for k_, v_, _ in chunks]
                                  + sliced + full_new)

                        expS = ep.tile([P, 16, TB], F16, name="expS", tag="expS")
                        sumP = sp_.tile([P, TB], F16, name="sumP", tag="sumP")
                        av = avp.tile([P, TB], F32, name="av", tag="av")

                        # scores run one chunk ahead of av on the PE so the
                        # exp (ACT) latency of chunk ci hides under the
                        # score matmul of chunk ci+1
                        pend_av = None
                        for ci, (k_ap, v_ap, ri, diag0) in enumerate(chunks):
                            c0 = 0 if ri is None else P * ri
                            s_ps = scp.tile([P, TB], F32, name="s_ps", tag="s")
                            nc.tensor.matmul(s_ps[:, c0:], k_ap, q_ap[:, c0:],
                                             start=True, stop=True)
                            if ri is not None or diag0:
                                nc.vector.tensor_add(s_ps[:, c0:c0 + P],
                                                     s_ps[:, c0:c0 + P], tri_t[:])
                            e_ap = expS[:, ci, c0:]
                            nc.scalar.activation(e_ap, s_ps[:, c0:], EXP,
                                                 bias=ebias[:])
                            if ci == 0:
                                nc.vector.tensor_copy(sumP[:], e_ap)
                            else:
                                nc.vector.tensor_add(sumP[:, c0:], sumP[:, c0:],
                                                     e_ap)
                            if pend_av is not None:
                                pv_ap, pe_ap, pc0, pci = pend_av
                                nc.tensor.matmul(av[:, pc0:], pv_ap, pe_ap,
                                                 start=(pci == 0), stop=False)
                            pend_av = (v_ap, e_ap, c0, ci)
                        pv_ap, pe_ap, pc0, pci = pend_av
                        nc.tensor.matmul(av[:, pc0:], pv_ap, pe_ap,
                                         start=False, stop=True)

                        # softmax denominator: partition-reduce + broadcast via
                        # ones matmul, then fast approx reciprocal
                        rbc = bcp.tile([P, TB], F32, name="rbc", tag="rbc")
                        nc.tensor.matmul(rbc[:], ones_t[:], sumP[:],
                                         start=True, stop=True)
                        rinv = aw.tile([P, TB], F32, name="rinv", tag="rinv")
                        nc.vector.reciprocal_approx_fast(rinv[:], rbc[:])
                        nc.vector.tensor_mul(att[:, t0:t0 + TB], av[:], rinv[:])

                    pend_out = t0
            emit_outproj(pend_out)


def build_nc():
    nc = bacc.Bacc("TRN2")
    xT = nc.dram_tensor("xT", [P, NB // TB, 16, TB], F16, kind="ExternalInput")
    wq = nc.dram_tensor("wq", [P, 16 * G * P], STAT, kind="ExternalInput")
    wk = nc.dram_tensor("wk", [P, 16 * P], STAT, kind="ExternalInput")
    wv = nc.dram_tensor("wv", [P, 16 * P], STAT, kind="ExternalInput")
    wo = nc.dram_tensor("wo", [P, G * C], F16, kind="ExternalInput")
    kpT = nc.dram_tensor("kpT", [P, B * 8 * P], STAT, kind="ExternalInput")
    vpn = nc.dram_tensor("vpn", [P, B * 8 * P], STAT, kind="ExternalInput")
    cosq = nc.dram_tensor("cosq", [P, T], F16, kind="ExternalInput")
    sinq = nc.dram_tensor("sinq", [P, T], F16, kind="ExternalInput")
    cosk = nc.dram_tensor("cosk", [P, T], F16, kind="ExternalInput")
    sink = nc.dram_tensor("sink", [P, T], F16, kind="ExternalInput")
    tri = nc.dram_tensor("tri", [P, P], F16, kind="ExternalInput")
    rperm = nc.dram_tensor("rperm", [P, P], STAT, kind="ExternalInput")
    ones = nc.dram_tensor("ones", [P, P], STAT, kind="ExternalInput")
    ident = nc.dram_tensor("ident", [P, P], STAT, kind="ExternalInput")
    y = nc.dram_tensor("y", [NB, C], F16, kind="ExternalOutput")
    io = (xT, wq, wk, wv, wo, kpT, vpn, cosq, sinq, cosk, sink, tri,
          rperm, ones, ident, y)
    with nc.allow_low_precision(reason="fp16 operands; fp32 accumulation"):
        with tile.TileContext(nc) as tc:
            _emit(tc, io)
    nc.compile()
    return nc


def host_inputs(x, Wq, Wkv, Wo, K_pool, V_pool, slot_map, past_len):
    x = np.asarray(x, dtype=np.float32)
    Wq = np.asarray(Wq, dtype=np.float32)
    Wkv = np.asarray(Wkv, dtype=np.float32)
    Wo = np.asarray(Wo, dtype=np.float32)
    K_pool = np.asarray(K_pool, dtype=np.float32)
    V_pool = np.asarray(V_pool, dtype=np.float32)
    slot_map = np.asarray(slot_map, dtype=np.int32)
    past = int(past_len)
    assert past == PAST, f"kernel hardcodes past_len={PAST}, got {past}"

    # [p, tb, kc, tok%512]: per-partition fully contiguous x tiles
    xT = (x.reshape(NB, C).T.reshape(16, P, NB // TB, TB)
          .transpose(1, 2, 0, 3))
    xT = np.ascontiguousarray(xT.astype(np.float16))

    # rope tables; argument arithmetic mirrors the f32 ops of the reference
    idx = np.arange(D // 2, dtype=np.float32)
    inv = np.float32(1.0) / np.float32(10000.0) ** (idx / np.float32(D // 2))
    inv = inv.astype(np.float32)
    t = np.arange(past, past + T, dtype=np.float32)
    freqs = (t[:, None] * inv[None, :]).astype(np.float32)
    emb = np.concatenate([freqs, freqs], axis=1)
    cos = np.cos(emb).astype(np.float32)
    sin = np.sin(emb).astype(np.float32)
    qscale = np.float32(1.0) / np.sqrt(np.float32(D))
    cosqT = np.ascontiguousarray((cos * qscale).T.astype(np.float16))
    sinqT = np.ascontiguousarray((sin * qscale).T.astype(np.float16))
    coskT = np.ascontiguousarray(cos.T.astype(np.float16))
    sinkT = np.ascontiguousarray(sin.T.astype(np.float16))

    # shared [128,128] triangular mask for block-aligned causal diagonals
    s_i = np.arange(P)[:, None]
    u_i = np.arange(P)[None, :]
    tri = np.where(s_i <= u_i, 0.0, NEG).astype(np.float16)

    rperm = np.zeros((P, P), np.float32)
    for d in range(D // 2):
        rperm[d + D // 2, d] = -1.0       # rot(q)[d] = -q[d+64] for d < 64
        rperm[d, d + D // 2] = 1.0        # rot(q)[d] = q[d-64] for d >= 64
    rperm = rperm.astype(NP_BF16)
    ones = np.ones((P, P), NP_BF16)
    ident = np.eye(P, dtype=np.float32).astype(NP_BF16)

    # host-side past-KV gather (+ transpose for K): logical past order
    gs = np.asarray(slot_map[:, :past], dtype=np.int64)     # [B, 1024]
    in_maps = []
    for c in range(NCORES):
        Kg = K_pool[gs, c, :].astype(NP_BF16)               # [B, 1024, 128]
        Vg = V_pool[gs, c, :].astype(NP_BF16)
        # kT_past [d, b, j, s%128]  -> flat [128, B*8*128]
        kpT = np.ascontiguousarray(
            Kg.reshape(B, 8, P, D).transpose(3, 0, 1, 2).reshape(P, B * 8 * P))
        # vg [s%128, b, j, d] -> flat [128, B*8*128]
        vpn = np.ascontiguousarray(
            Vg.reshape(B, 8, P, D).transpose(2, 0, 1, 3).reshape(P, B * 8 * P))
        # weight tiles pre-arranged to [partition, kc*m] so device loads are
        # one contiguous run per partition
        wq_l = Wq[G * D * c:G * D * (c + 1), :].T.reshape(16, P, G * D)
        wq_l = wq_l.transpose(1, 0, 2).reshape(P, 16 * G * D)
        wk_l = Wkv[D * c:D * (c + 1), :].T.reshape(16, P, D)
        wk_l = wk_l.transpose(1, 0, 2).reshape(P, 16 * D)
        wv_l = Wkv[HKV * D + D * c:HKV * D + D * (c + 1), :].T.reshape(16, P, D)
        wv_l = wv_l.transpose(1, 0, 2).reshape(P, 16 * D)
        wo_l = Wo[:, G * D * c:G * D * (c + 1)].T.reshape(G, P, C)
        wo_l = wo_l.transpose(1, 0, 2).reshape(P, G * C)
        in_maps.append({
            "xT": xT,
            "wq": np.ascontiguousarray(wq_l.astype(NP_BF16)),
            "wk": np.ascontiguousarray(wk_l.astype(NP_BF16)),
            "wv": np.ascontiguousarray(wv_l.astype(NP_BF16)),
            "wo": np.ascontiguousarray(wo_l.astype(np.float16)),
            "kpT": kpT, "vpn": vpn,
            "cosq": cosqT, "sinq": sinqT, "cosk": coskT, "sink": sinkT,
            "tri": tri, "rperm": rperm, "ones": ones, "ident": ident,
        })
    return in_maps


_NC_CACHE = None


def kernel(**inputs):
    global _NC_CACHE
    in_maps = host_inputs(**inputs)
    if _NC_CACHE is None:
        _NC_CACHE = build_nc()
    res = run_bass_kernel_spmd(_NC_CACHE, in_maps, core_ids=list(range(NCORES)))
    y = res.results[0]["y"].astype(np.float32)
    for c in range(1, NCORES):
        y = y + res.results[c]["y"].astype(np.float32)
    return y.reshape(B, T, C)


# revision 44
# speedup vs baseline: 1.1552x; 1.1552x over previous
"""Trainium2 Bass kernel for paged causal self-attention (GQA + YaRN rope).

Sharding: tensor-parallel over heads. Core c (of 8) owns kv-head c and
q-heads 2c, 2c+1 for both batches. Each core computes a partial output
y_c = attn_c @ Wo_c.T over its 256 channels; the host sums the 8 partials.

The reference's scatter of new K/V into the pools is dead code w.r.t. the
returned output; new K/V are consumed directly from SBUF. The past-KV
gather (slot_map indexed) and the [s,d]/[d,s] layout transposes are done
on the host, so the device sees two contiguous fp16 layouts.

All matmul operands are fp16 (host-precast); accumulation stays fp32 in
PSUM. exp uses a constant -4 bias (cancels in softmax) for fp16 range
margin. Causal diagonal chunks are column-sliced and share one [128,128]
triangular mask tile.
"""

import sys

sys.path.insert(0, "/opt/trn_rl_repo")

import ml_dtypes
import numpy as np

NP_BF16 = np.dtype(ml_dtypes.bfloat16)

import concourse.bacc as bacc
import concourse.bass as bass
import concourse.tile as tile
from concourse import mybir
from concourse.bass_utils import run_bass_kernel_spmd

F32 = mybir.dt.float32
F16 = mybir.dt.float16
BF16 = mybir.dt.bfloat16
# stationary matmul operands use bf16 (enables fast weight load);
# moving operands stay fp16 for precision
STAT = BF16
EXP = mybir.ActivationFunctionType.Exp

B, T, PAST = 2, 1024, 1024
H, HKV, D = 16, 8, 128
G = H // HKV            # q heads per kv head
C = H * D               # 2048
TOTAL = PAST + T        # 2048
NB = B * T              # 2048 flattened tokens
NCORES = 8
P = 128
TB = 512                # token block
NEG = -60000.0          # mask value (fp16-representable; exp underflows to 0)
EBIAS = -4.0            # constant exp bias; cancels in softmax


def _emit(tc, io):
    nc = tc.nc
    (xT, wq, wk, wv, wo, kpT, vpn, cosq, sinq, cosk, sink, tri, rperm,
     ones, ident, y) = io

    with (
        tc.tile_pool(name="const", bufs=1) as cp,
        tc.tile_pool(name="persist", bufs=1) as pp,
        tc.tile_pool(name="ysb", bufs=4) as yp,
    ):
        # ---- weight tiles (loaded chunked, interleaved with the first x
        # tiles, inside the phase-1 loop) ----
        wq_t = pp.tile([P, 16, G * P], STAT)
        wk_t = pp.tile([P, 16, P], STAT)
        wv_t = pp.tile([P, 16, P], STAT)

        ebias = cp.tile([P, 1], F32)
        nc.vector.memset(ebias[:], EBIAS)

        # ---- remaining constant tiles; their loads are emitted inside the
        # first phase-1 block (SWDGE on the idle GpSimd queue, dep-delayed
        # behind the last weight chunk so they don't steal DMA bandwidth
        # from the startup-critical loads) ----
        gdep = cp.tile([P, 8], STAT)
        cosq_t = pp.tile([P, T], F16)
        sinq_t = pp.tile([P, T], F16)
        cosk_t = pp.tile([P, T], F16)
        sink_t = pp.tile([P, T], F16)
        tri_t = cp.tile([P, P], F16)
        rperm_t = cp.tile([P, P], STAT)
        ones_t = cp.tile([P, P], STAT)
        ident_t = cp.tile([P, P], STAT)
        kT_past = pp.tile([P, B, 8, P], STAT)  # [d, b, chunk, s%128]
        vg = pp.tile([P, B, 8, P], STAT)       # [s%128, b, chunk, d]
        wo_t = pp.tile([P, G, C], F16)

        gdep2 = cp.tile([P, 8], STAT)

        def emit_const_loads_a():
            # phase-1 constants (rope tables etc.) — gated behind the last
            # weight chunk so they don't contend with the startup loads
            nc.gpsimd.tensor_copy(gdep[:], wq_t[:, 15, 0:8])
            nc.gpsimd.dma_start(cosq_t[:], cosq[:])
            nc.gpsimd.dma_start(sinq_t[:], sinq[:])
            nc.gpsimd.dma_start(cosk_t[:], cosk[:])
            nc.gpsimd.dma_start(sink_t[:], sink[:])
            nc.gpsimd.dma_start(tri_t[:], tri[:])
            nc.gpsimd.dma_start(rperm_t[:], rperm[:])
            nc.gpsimd.dma_start(ones_t[:], ones[:])
            nc.gpsimd.dma_start(ident_t[:], ident[:])

        def emit_const_loads_b(gate):
            # phase-2 constants — gated to the third projection block so
            # their 2MB doesn't clog the DMA rings during phase-1 streaming
            nc.gpsimd.tensor_copy(gdep2[:], gate[:, 0:8])
            nc.gpsimd.dma_start(kT_past[:],
                                kpT.rearrange("p (b j m) -> p b j m", b=B, j=8))
            nc.gpsimd.dma_start(vg[:],
                                vpn.rearrange("p (b j m) -> p b j m", b=B, j=8))
            nc.gpsimd.dma_start(wo_t[:], wo.rearrange("p (g m) -> p g m", g=G))

        # ---- persistent activations ----
        qT0 = pp.tile([P, NB], F16)       # q head 2c,   [d, token]  (moving)
        qT1 = pp.tile([P, NB], F16)       # q head 2c+1
        kT_new = pp.tile([P, NB], STAT)   # new keys,    [d, token]  (stationary)
        v_nat = pp.tile([P, B, 8, P], STAT)    # new values, [t%128, b, chunk, d]
        att0 = pp.tile([P, NB], STAT)     # attention out head 2c, [d, token]
        att1 = pp.tile([P, NB], STAT)

        # ================= phase 1: projections + rope =================
        with (
            tc.tile_pool(name="xt", bufs=8) as xp,
            tc.tile_pool(name="rope", bufs=3) as rp,
            tc.tile_pool(name="pproj", bufs=1, space="PSUM") as pjp,
            tc.tile_pool(name="prope", bufs=2, space="PSUM") as rpp,
            tc.tile_pool(name="ptr", bufs=1, space="PSUM") as trp,
        ):
            # Rope/v-transpose emission is deferred one block: the PE's
            # in-order stream gets block N+1's projection matmuls BEFORE
            # block N's rope matmuls, so PSUM-evacuation copies (ACT/DVE)
            # have a full block of slack to land.
            pending = None

            def emit_rope(state):
                tbp, raw_k_, raw_q0_, raw_q1_, vsb_ = state
                n0p = tbp * TB
                bp = tbp // 2
                tposp = (tbp % 2) * TB
                # v transposes first (their input was evacuated earliest);
                # all four land in one PSUM tile, one batched evacuation
                vtp = trp.tile([P, 4, P], STAT, name="vtp", tag="vtp")
                for j4 in range(TB // P):
                    nc.tensor.transpose(vtp[:, j4, :],
                                        vsb_[:, j4 * P:(j4 + 1) * P], ident_t[:])
                # evacuate the transposes first so their PSUM bank (reused by
                # phase-2 pools) is released before the mul chain drains
                nc.vector.tensor_copy(
                    v_nat[:, bp, (tbp % 2) * 4:(tbp % 2) * 4 + 4, :], vtp[:])
                for raw, dst, ct, stt in (
                    (raw_k_, kT_new, cosk_t, sink_t),
                    (raw_q0_, qT0, cosq_t, sinq_t),
                    (raw_q1_, qT1, cosq_t, sinq_t),
                ):
                    rot = rpp.tile([P, TB], F32, name="rot", tag="rot")
                    nc.tensor.matmul(rot[:], rperm_t[:], raw[:], start=True,
                                     stop=True)
                    t1 = rp.tile([P, TB], F16, name="t1", tag="t1")
                    nc.vector.tensor_mul(t1[:], raw[:], ct[:, tposp:tposp + TB])
                    t2 = rp.tile([P, TB], F16, name="t2", tag="t2")
                    nc.vector.tensor_mul(t2[:], rot[:], stt[:, tposp:tposp + TB])
                    nc.vector.tensor_add(dst[:, n0p:n0p + TB], t1[:], t2[:])

            # block order (2,3,0,1): the last-roped block (1) is only needed
            # by the final attention groups, so phase 2 starts without
            # waiting for the phase-1 tail.
            for ti, tb in enumerate((2, 3, 0, 1)):
                n0 = tb * TB

                # previous block's q psum evacuations (ACT queue, ready now)
                if pending is not None:
                    tbp, q0p_, q1p_, raw_k_, vsb_ = pending
                    raw_q0 = rp.tile([P, TB], F16, name="raw_q0", tag="rawq0")
                    nc.scalar.copy(raw_q0[:], q0p_[:])
                    raw_q1 = rp.tile([P, TB], F16, name="raw_q1", tag="rawq1")
                    nc.scalar.copy(raw_q1[:], q1p_[:])
                    pending = (tbp, raw_k_, raw_q0, raw_q1, vsb_)

                q0p = pjp.tile([P, TB], F32, name="q0p", tag="q0")
                q1p = pjp.tile([P, TB], F32, name="q1p", tag="q1")
                kkp = pjp.tile([P, TB], F32, name="kkp", tag="kk")
                vvp = pjp.tile([P, TB], F32, name="vvp", tag="vv")
                for kcg in range(4):
                    if ti == 0:
                        # first block: interleave the weight-chunk loads with
                        # the x stream so the first matmul starts early
                        nc.sync.dma_start(
                            wq_t[:, 4 * kcg:4 * kcg + 4, :],
                            wq[:, kcg * 4 * G * P:(kcg + 1) * 4 * G * P]
                            .rearrange("p (kc m) -> p kc m", kc=4))
                        nc.sync.dma_start(
                            wk_t[:, 4 * kcg:4 * kcg + 4, :],
                            wk[:, kcg * 4 * P:(kcg + 1) * 4 * P]
                            .rearrange("p (kc m) -> p kc m", kc=4))
                        nc.sync.dma_start(
                            wv_t[:, 4 * kcg:4 * kcg + 4, :],
                            wv[:, kcg * 4 * P:(kcg + 1) * 4 * P]
                            .rearrange("p (kc m) -> p kc m", kc=4))
                    xt4 = xp.tile([P, 4, TB], F16, name="xt4", tag="xt")
                    nc.sync.dma_start(xt4[:], xT[:, tb, 4 * kcg:4 * kcg + 4, :])
                    for kc4 in range(4):
                        kc = 4 * kcg + kc4
                        xt = xt4[:, kc4, :]
                        st = (kc == 0)
                        sp = (kc == 15)
                        nc.tensor.matmul(q0p[:], wq_t[:, kc, 0:P], xt, start=st, stop=sp)
                        nc.tensor.matmul(q1p[:], wq_t[:, kc, P:2 * P], xt, start=st, stop=sp)
                        nc.tensor.matmul(kkp[:], wk_t[:, kc, :], xt, start=st, stop=sp)
                        nc.tensor.matmul(vvp[:], wv_t[:, kc, :], xt, start=st, stop=sp)

                # evacuate kk/vv immediately on separate engines
                raw_k = rp.tile([P, TB], F16, name="raw_k", tag="rawk")
                nc.scalar.copy(raw_k[:], kkp[:])
                vsb = rp.tile([P, TB], STAT, name="vsb", tag="vsb")
                nc.vector.tensor_copy(vsb[:], vvp[:])

                if ti == 0:
                    emit_const_loads_a()
                elif ti == 2:
                    emit_const_loads_b(raw_k)
                if pending is not None:
                    emit_rope(pending)
                pending = (tb, q0p, q1p, raw_k, vsb)

            # drain the final block (q0/q1 evacuations on separate engines —
            # no later projection matmuls will hide their latency)
            tbp, q0p_, q1p_, raw_k_, vsb_ = pending
            raw_q0 = rp.tile([P, TB], F16, name="raw_q0", tag="rawq0")
            nc.scalar.copy(raw_q0[:], q0p_[:])
            raw_q1 = rp.tile([P, TB], F16, name="raw_q1", tag="rawq1")
            nc.vector.tensor_copy(raw_q1[:], q1p_[:])
            emit_rope((tbp, raw_k_, raw_q0, raw_q1, vsb_))

        # ================= phase 2+3: attention + output proj =================
        with (
            tc.tile_pool(name="exps", bufs=2) as ep,
            tc.tile_pool(name="sums", bufs=2) as sp_,
            tc.tile_pool(name="attw", bufs=2) as aw,
            tc.tile_pool(name="pscore", bufs=3, space="PSUM") as scp,
            tc.tile_pool(name="pav", bufs=2, space="PSUM") as avp,
            tc.tile_pool(name="pbc", bufs=1, space="PSUM") as bcp,
            tc.tile_pool(name="py", bufs=2, space="PSUM") as pyp,
        ):
            def emit_outproj(t0_):
                # output projection for 512 tokens; evacuate the four 512-ch
                # chunks into one wide tile, one contiguous DMA per 128 toks
                for tc4 in range(4):
                    tt0 = t0_ + tc4 * P
                    ysb = yp.tile([P, 4, TB], F16, name="ysbt", tag="ysbt")
                    for cb in range(4):
                        yps = pyp.tile([P, TB], F32, name="yps", tag="y")
                        nc.tensor.matmul(yps[:], att0[:, tt0:tt0 + P],
                                         wo_t[:, 0, cb * TB:(cb + 1) * TB],
                                         start=True, stop=False)
                        nc.tensor.matmul(yps[:], att1[:, tt0:tt0 + P],
                                         wo_t[:, 1, cb * TB:(cb + 1) * TB],
                                         start=False, stop=True)
                        if (tc4 + cb) % 2 == 0:
                            nc.scalar.copy(ysb[:, cb, :], yps[:])
                        else:
                            nc.vector.tensor_copy(ysb[:, cb, :], yps[:])
                    nc.sync.dma_start(y[tt0:tt0 + P, :], ysb[:])

            pend_out = None
            for b in range(B):
                for tbq in range(2):             # query block of 512 in batch
                    t0 = b * T + tbq * TB
                    for g, (qT, att) in enumerate(((qT0, att0), (qT1, att1))):
                        if g == 1 and pend_out is not None:
                            # previous block's output projection, emitted
                            # here so its att inputs have a chunk-loop of
                            # slack and its matmuls fill exp-latency bubbles
                            emit_outproj(pend_out)
                            pend_out = None
                        q_ap = qT[:, t0:t0 + TB]
                        njnew = 4 * tbq + 4
                        nch = 8 + njnew

                        # chunk list: past (full), sliced-diag new (desc ri),
                        # then full new; last chunk is full-width.
                        chunks = [(kT_past[:, b, j, :], vg[:, b, j, :], None)
                                  for j in range(8)]
                        sliced = []
                        full_new = []
                        for j in range(njnew):
                            koff = b * T + j * P
                            ri = j - 4 * tbq
                            ent = (kT_new[:, koff:koff + P],
                                   v_nat[:, b, j, :], ri if ri > 0 else None,
                                   ri == 0)
                            if ri > 0:
                                sliced.append(ent)
                            else:
                                full_new.append(ent)
                        sliced.reverse()   # descending ri
                        # order: past, sliced (ri 3..1), full new (ri<0), ri==0 last
                        full_new.sort(key=lambda e: e[3])
                        chunks = ([(k_, v_, None, False) for k_, v_, _ in chunks]
                                  + sliced + full_new)

                        expS = ep.tile([P, 16, TB], F16, name="expS", tag="expS")
                        sumP = sp_.tile([P, TB], F16, name="sumP", tag="sumP")
                        av = avp.tile([P, TB], F32, name="av", tag="av")

                        # scores run one chunk ahead of av on the PE so the
                        # exp (ACT) latency of chunk ci hides under the
                        # score matmul of chunk ci+1
                        pend_av = None
                        for ci, (k_ap, v_ap, ri, diag0) in enumerate(chunks):
                            c0 = 0 if ri is None else P * ri
                            s_ps = scp.tile([P, TB], F32, name="s_ps", tag="s")
                            nc.tensor.matmul(s_ps[:, c0:], k_ap, q_ap[:, c0:],
                                             start=True, stop=True)
                            if ri is not None or diag0:
                                nc.vector.tensor_add(s_ps[:, c0:c0 + P],
                                                     s_ps[:, c0:c0 + P], tri_t[:])
                            e_ap = expS[:, ci, c0:]
                            nc.scalar.activation(e_ap, s_ps[:, c0:], EXP,
                                                 bias=ebias[:])
                            if ci == 0:
                                nc.vector.tensor_copy(sumP[:], e_ap)
                            else:
                                nc.vector.tensor_add(sumP[:, c0:], sumP[:, c0:],
                                                     e_ap)
                            if pend_av is not None:
                                pv_ap, pe_ap, pc0, pci = pend_av
                                nc.tensor.matmul(av[:, pc0:], pv_ap, pe_ap,
                                                 start=(pci == 0), stop=False)
                            pend_av = (v_ap, e_ap, c0, ci)
                        pv_ap, pe_ap, pc0, pci = pend_av
                        nc.tensor.matmul(av[:, pc0:], pv_ap, pe_ap,
                                         start=False, stop=True)

                        # softmax denominator: partition-reduce + broadcast via
                        # ones matmul, then fast approx reciprocal
                        rbc = bcp.tile([P, TB], F32, name="rbc", tag="rbc")
                        nc.tensor.matmul(rbc[:], ones_t[:], sumP[:],
                                         start=True, stop=True)
                        rinv = aw.tile([P, TB], F32, name="rinv", tag="rinv")
                        nc.vector.reciprocal_approx_fast(rinv[:], rbc[:])
                        nc.vector.tensor_mul(att[:, t0:t0 + TB], av[:], rinv[:])

                    pend_out = t0
            emit_outproj(pend_out)


def build_nc():
    nc = bacc.Bacc("TRN2")
    xT = nc.dram_tensor("xT", [P, NB // TB, 16, TB], F16, kind="ExternalInput")
    wq = nc.dram_tensor("wq", [P, 16 * G * P], STAT, kind="ExternalInput")
    wk = nc.dram_tensor("wk", [P, 16 * P], STAT, kind="ExternalInput")
    wv = nc.dram_tensor("wv", [P, 16 * P], STAT, kind="ExternalInput")
    wo = nc.dram_tensor("wo", [P, G * C], F16, kind="ExternalInput")
    kpT = nc.dram_tensor("kpT", [P, B * 8 * P], STAT, kind="ExternalInput")
    vpn = nc.dram_tensor("vpn", [P, B * 8 * P], STAT, kind="ExternalInput")
    cosq = nc.dram_tensor("cosq", [P, T], F16, kind="ExternalInput")
    sinq = nc.dram_tensor("sinq", [P, T], F16, kind="ExternalInput")
    cosk = nc.dram_tensor("cosk", [P, T], F16, kind="ExternalInput")
    sink = nc.dram_tensor("sink", [P, T], F16, kind="ExternalInput")
    tri = nc.dram_tensor("tri", [P, P], F16, kind="ExternalInput")
    rperm = nc.dram_tensor("rperm", [P, P], STAT, kind="ExternalInput")
    ones = nc.dram_tensor("ones", [P, P], STAT, kind="ExternalInput")
    ident = nc.dram_tensor("ident", [P, P], STAT, kind="ExternalInput")
    y = nc.dram_tensor("y", [NB, C], F16, kind="ExternalOutput")
    io = (xT, wq, wk, wv, wo, kpT, vpn, cosq, sinq, cosk, sink, tri,
          rperm, ones, ident, y)
    with nc.allow_low_precision(reason="fp16 operands; fp32 accumulation"):
        with tile.TileContext(nc) as tc:
            _emit(tc, io)
    nc.compile()
    return nc


def host_inputs(x, Wq, Wkv, Wo, K_pool, V_pool, slot_map, past_len):
    x = np.asarray(x, dtype=np.float32)
    Wq = np.asarray(Wq, dtype=np.float32)
    Wkv = np.asarray(Wkv, dtype=np.float32)
    Wo = np.asarray(Wo, dtype=np.float32)
    K_pool = np.asarray(K_pool, dtype=np.float32)
    V_pool = np.asarray(V_pool, dtype=np.float32)
    slot_map = np.asarray(slot_map, dtype=np.int32)
    past = int(past_len)
    assert past == PAST, f"kernel hardcodes past_len={PAST}, got {past}"

    # [p, tb, kc, tok%512]: per-partition fully contiguous x tiles
    xT = (x.reshape(NB, C).T.reshape(16, P, NB // TB, TB)
          .transpose(1, 2, 0, 3))
    xT = np.ascontiguousarray(xT.astype(np.float16))

    # rope tables; argument arithmetic mirrors the f32 ops of the reference
    idx = np.arange(D // 2, dtype=np.float32)
    inv = np.float32(1.0) / np.float32(10000.0) ** (idx / np.float32(D // 2))
    inv = inv.astype(np.float32)
    t = np.arange(past, past + T, dtype=np.float32)
    freqs = (t[:, None] * inv[None, :]).astype(np.float32)
    emb = np.concatenate([freqs, freqs], axis=1)
    cos = np.cos(emb).astype(np.float32)
    sin = np.sin(emb).astype(np.float32)
    qscale = np.float32(1.0) / np.sqrt(np.float32(D))
    cosqT = np.ascontiguousarray((cos * qscale).T.astype(np.float16))
    sinqT = np.ascontiguousarray((sin * qscale).T.astype(np.float16))
    coskT = np.ascontiguousarray(cos.T.astype(np.float16))
    sinkT = np.ascontiguousarray(sin.T.astype(np.float16))

    # shared [128,128] triangular mask for block-aligned causal diagonals
    s_i = np.arange(P)[:, None]
    u_i = np.arange(P)[None, :]
    tri = np.where(s_i <= u_i, 0.0, NEG).astype(np.float16)

    rperm = np.zeros((P, P), np.float32)
    for d in range(D // 2):
        rperm[d + D // 2, d] = -1.0       # rot(q)[d] = -q[d+64] for d < 64
        rperm[d, d + D // 2] = 1.0        # rot(q)[d] = q[d-64] for d >= 64
    rperm = rperm.astype(NP_BF16)
    ones = np.ones((P, P), NP_BF16)
    ident = np.eye(P, dtype=np.float32).astype(NP_BF16)

    # host-side past-KV gather (+ transpose for K): logical past order
    gs = np.asarray(slot_map[:, :past], dtype=np.int64)     # [B, 1024]
    in_maps = []
    for c in range(NCORES):
        Kg = K_pool[gs, c, :].astype(NP_BF16)               # [B, 1024, 128]
        Vg = V_pool[gs, c, :].astype(NP_BF16)
        # kT_past [d, b, j, s%128]  -> flat [128, B*8*128]
        kpT = np.ascontiguousarray(
            Kg.reshape(B, 8, P, D).transpose(3, 0, 1, 2).reshape(P, B * 8 * P))
        # vg [s%128, b, j, d] -> flat [128, B*8*128]
        vpn = np.ascontiguousarray(
            Vg.reshape(B, 8, P, D).transpose(2, 0, 1, 3).reshape(P, B * 8 * P))
        # weight tiles pre-arranged to [partition, kc*m] so device loads are
        # one contiguous run per partition
        wq_l = Wq[G * D * c:G * D * (c + 1), :].T.reshape(16, P, G * D)
        wq_l = wq_l.transpose(1, 0, 2).reshape(P, 16 * G * D)
        wk_l = Wkv[D * c:D * (c + 1), :].T.reshape(16, P, D)
        wk_l = wk_l.transpose(1, 0, 2).reshape(P, 16 * D)
        wv_l = Wkv[HKV * D + D * c:HKV * D + D * (c + 1), :].T.reshape(16, P, D)
        wv_l = wv_l.transpose(1, 0, 2).reshape(P, 16 * D)
        wo_l = Wo[:, G * D * c:G * D * (c + 1)].T.reshape(G, P, C)
        wo_l = wo_l.transpose(1, 0, 2).reshape(P, G * C)
        in_maps.append({
            "xT": xT,
            "wq": np.ascontiguousarray(wq_l.astype(NP_BF16)),
            "wk": np.ascontiguousarray(wk_l.astype(NP_BF16)),
            "wv": np.ascontiguousarray(wv_l.astype(NP_BF16)),
            "wo": np.ascontiguousarray(wo_l.astype(np.float16)),
            "kpT": kpT, "vpn": vpn,
            "cosq": cosqT, "sinq": sinqT, "cosk": coskT, "sink": sinkT,
            "tri": tri, "rperm": rperm, "ones": ones, "ident": ident,
        })
    return in_maps


_NC_CACHE = None


def kernel(**inputs):
    global _NC_CACHE
    in_maps = host_inputs(**inputs)
    if _NC_CACHE is None:
        _NC_CACHE = build_nc()
    res = run_bass_kernel_spmd(_NC_CACHE, in_maps, core_ids=list(range(NCORES)))
    y = res.results[0]["y"].astype(np.float32)
    for c in range(1, NCORES):
        y = y + res.results[c]["y"].astype(np.float32)
    return y.reshape(B, T, C)
